# revision 1
# baseline (speedup 1.0000x reference)
"""Trainium2 Bass kernel for nn_LIF_hh_neuron (B=2048, T=15, IN=512, C=1024).

Sharding: pure data-parallel over batch B across 8 NeuronCores (256 each).

Per core:
  - p_k(t) = x_t @ W_k.T runs on PE as a 3-pass hi/lo split for fp32-class
    precision at full PE rate: xh(f32r)@Wh(f32r) + xh(bf16)@Wl(bf16) +
    xl(f32r)@Wh(f32r), where h/l split W and x on the f32r (11-bit
    mantissa) grid.  Measured max abs err ~5e-7 vs 2e-4 for 1-pass f32r.
  - LIF state update per timestep stays on-chip:
      mem(t)   = psum(t) + upb(t)                  [DVE tensor_tensor]
      spike(t) = mem(t) > 0.8 -> interleaved (c,j) [DVE tensor_scalar]
      upb(t+1) = select(mem<=0.8, 0.2*mem, 0) + b  [custom DVE op]
      v_w      = wl_w*mem_w (+bl for w=0)          [ACT scaled copies]
      z(t+1)   = v0+v1+v2                          [GPSIMD adds]
      plane 3 (no matmul): s3=(z+u3)>0.8, u3'=gate [2-src custom DVE ops]
"""

import numpy as np
import ml_dtypes

import concourse.bass as bass
import concourse.mybir as mybir
import concourse.tile as tile
from concourse import bacc
from concourse import bass_utils
from concourse.dve_spec import Spec, Src0, Src1, C0, C1, Zero, select, lower
from concourse.dve_ops import has_src1, DveOp, OPS
import concourse.dve_ops as dve_ops_mod
from concourse.dve_uop import DveOpSpec

F32 = mybir.dt.float32
F32R = mybir.dt.float32r
BF16 = mybir.dt.bfloat16

B, T, IN, C = 2048, 15, 512, 1024
NCORES = 8
BLOC = B // NCORES          # 256 batches per core
NBT = BLOC // 128           # 2 batch tiles per core
KC = IN // 128              # 4 contraction chunks
NH = 2                      # two c-halves of 512
THRESH = 0.8
DECAY = 0.2


def _register_op(name, spec, subdim=False):
    for existing in OPS:
        if existing.name == name:
            return existing
    op = DveOp(name, spec, subdim=subdim, uops_sha={})
    OPS.append(op)
    dve_ops_mod._SUB_OPCODE_FOR_NAME[name] = (
        dve_ops_mod._CUSTOM_DVE_ROW_BASE + len(OPS) - 1
    )
    dve_ops_mod.CUSTOM_DVE_SPECS[name] = spec
    shas = {}
    for ver in ("v3", "v4"):
        s = DveOpSpec(
            name=name,
            opcode=dve_ops_mod.get_dve_sub_opcode(name),
            uops=lower(spec, ver=ver),
            rd1_en=has_src1(spec),
        )
        shas[ver] = s.sha(ver)
    object.__setattr__(op, "uops_sha", shas)
    return op


# upb = select(mem <= thr, mem*decay, 0) + b
LIF_UPB = _register_op(
    "LIF_UPB_ANT",
    Spec(
        body=select(Src0 <= C0, Src0 * C1, Zero) + Src1,
        reference=lambda in0, in1, s0, s1: (
            np.where(in0 <= s0, in0 * s1, 0.0) + in1
        ).astype(np.float32),
    ),
)

# s3 = (z + u3) > thr
LIF_SPIKE2 = _register_op(
    "LIF_SPIKE2_ANT",
    Spec(
        body=(Src0 + Src1) > C0,
        reference=lambda in0, in1, s0, s1: ((in0 + in1) > s0).astype(np.float32),
    ),
)

# u3' = select(z + u3 <= thr, (z + u3)*decay, 0)
LIF_GATE2 = _register_op(
    "LIF_GATE2_ANT",
    Spec(
        body=select((Src0 + Src1) <= C0, (Src0 + Src1) * C1, Zero),
        reference=lambda in0, in1, s0, s1: np.where(
            (in0 + in1) <= s0, (in0 + in1) * s1, 0.0
        ).astype(np.float32),
    ),
)


def _round11(a):
    """Round fp32 mantissa to 11 explicit bits (the f32r grid), nearest-even."""
    u = np.ascontiguousarray(a, np.float32).view(np.uint32)
    half = np.uint32(1 << 11)
    mask = np.uint32((1 << 12) - 1)
    frac = u & mask
    u2 = u & ~mask
    rup = (frac > half) | (
        (frac == half) & ((u2 >> np.uint32(12)) & np.uint32(1)).astype(bool)
    )
    return (u2 + np.where(rup, np.uint32(1 << 12), np.uint32(0))).view(np.float32)


def _build(passes=3):
    nc = bacc.Bacc("TRN2", target_bir_lowering=False, debug=False)

    # pre-transposed, pre-split x: [T, NBT, KC, 128(k), 128(b)]
    d_xhT = nc.dram_tensor("xhT", [T, NBT, KC, 128, 128], F32, kind="ExternalInput").ap()
    d_xh16T = nc.dram_tensor(
        "xh16T", [T, NBT, KC, 128, 128], BF16, kind="ExternalInput"
    ).ap()
    d_xlT = nc.dram_tensor("xlT", [T, NBT, KC, 128, 128], F32, kind="ExternalInput").ap()
    d_wh = nc.dram_tensor("wh", [KC, 128, 3, C], F32, kind="ExternalInput").ap()
    d_wl16 = nc.dram_tensor("wl16", [KC, 128, 3, C], BF16, kind="ExternalInput").ap()
    d_b = nc.dram_tensor("b", [1, 3, C], F32, kind="ExternalInput").ap()
    d_wlb = nc.dram_tensor("wlb", [1, 4], F32, kind="ExternalInput").ap()
    d_out = nc.dram_tensor("spk", [BLOC, T, 4 * C], F32, kind="ExternalOutput").ap()

    nchain = NBT * NH

    with tile.TileContext(nc) as tc:
        with (
            tc.tile_pool(name="wpool", bufs=1) as wpool,
            tc.tile_pool(name="state", bufs=1) as state,
            tc.tile_pool(name="mem", bufs=1) as mempool,
            tc.tile_pool(name="upb", bufs=1) as upbpool,
            tc.tile_pool(name="vpool", bufs=1) as vpool,
            tc.tile_pool(name="spool", bufs=2) as spool,
            tc.tile_pool(name="xin", bufs=2) as xin,
            tc.tile_pool(name="pspool", bufs=2, space="PSUM") as pspool,
        ):
            # ---- static tiles ----
            t_wh = wpool.tile([128, KC, 3, C], F32R, tag="wh")
            nc.sync.dma_start(
                out=t_wh, in_=d_wh.rearrange("k p w c -> p k w c").bitcast(F32R)
            )
            t_wl16 = wpool.tile([128, KC, 3, C], BF16, tag="wl16")
            nc.sync.dma_start(out=t_wl16, in_=d_wl16.rearrange("k p w c -> p k w c"))
            t_b = wpool.tile([128, NH, 3, 512], F32, tag="b")
            for h in range(NH):
                nc.sync.dma_start(
                    out=t_b[:, h],
                    in_=bass.AP(
                        tensor=d_b.tensor,
                        offset=h * 512,
                        ap=[[0, 128], [C, 3], [1, 512]],
                    ),
                )
            t_wlb = wpool.tile([128, 4], F32, tag="wlb")
            nc.sync.dma_start(
                out=t_wlb,
                in_=bass.AP(tensor=d_wlb.tensor, offset=0, ap=[[0, 128], [1, 4]]),
            )

            # ---- per-bt recurrent state ----
            t_z = [
                state.tile([128, NH, 512], F32, tag=f"z{bt}", name=f"z{bt}")
                for bt in range(NBT)
            ]
            t_u3 = [
                state.tile([128, NH, 512], F32, tag=f"u3{bt}", name=f"u3{bt}")
                for bt in range(NBT)
            ]
            for bt in range(NBT):
                nc.vector.memset(t_u3[bt], 0.0)
                nc.scalar.activation(
                    t_z[bt],
                    t_u3[bt],
                    mybir.ActivationFunctionType.Identity,
                    bias=t_wlb[:, 3:4],
                    scale=1.0,
                )

            upb_prev = [None] * NBT  # per-bt [128, 2*3, 512]; None -> b at t=0

            for t in range(T):
                for bt in range(NBT):
                    b0 = bt * 128
                    xhT = xin.tile([128, KC, 128], F32R, tag="xhT")
                    nc.sync.dma_start(
                        out=xhT,
                        in_=d_xhT[t, bt].rearrange("k p b -> p k b").bitcast(F32R),
                    )
                    xhT16 = xin.tile([128, KC, 128], BF16, tag="xhT16")
                    nc.sync.dma_start(
                        out=xhT16, in_=d_xh16T[t, bt].rearrange("k p b -> p k b")
                    )
                    xlT = xin.tile([128, KC, 128], F32R, tag="xlT")
                    nc.sync.dma_start(
                        out=xlT,
                        in_=d_xlT[t, bt].rearrange("k p b -> p k b").bitcast(F32R),
                    )

                    ps = [None] * NH
                    for h in range(NH):
                        ps[h] = pspool.tile(
                            [128, 3, 512], F32, tag="ps", name=f"ps_{t}_{bt}_{h}"
                        )
                    plist = [(xhT, t_wh), (xhT16, t_wl16), (xlT, t_wh)]
                    if passes == 1:
                        plist = plist[:1]
                    np_ = len(plist)
                    for k in range(KC):
                        for pi, (lhs, w_t) in enumerate(plist):
                            for h in range(NH):
                                c0 = h * 512
                                for w in range(3):
                                    nc.tensor.matmul(
                                        ps[h][:, w, :],
                                        lhs[:, k, :],
                                        w_t[:, k, w, c0 : c0 + 512],
                                        start=(k == 0 and pi == 0),
                                        stop=(k == KC - 1 and pi == np_ - 1),
                                    )

                    # mem(t) = psum + upb(t)   [per half]
                    mem_bt = mempool.tile(
                        [128, NH, 3, 512], F32, tag=f"mem{bt}", name=f"mem{bt}_{t}"
                    )
                    ub = upb_prev[bt]
                    for h in range(NH):
                        if ub is None:
                            ub_h = t_b[:, h]
                        else:
                            ub_h = ub[:, h * 3 : (h + 1) * 3, :]
                        nc.vector.tensor_tensor(
                            out=mem_bt[:, h],
                            in0=ps[h],
                            in1=ub_h,
                            op=mybir.AluOpType.add,
                        )

                    # spikes for all 4 planes, interleaved (c', j) in one tile
                    S = spool.tile([128, 2 * 512, 4], F32, tag="S")
                    # planes 0-2: read-strided over (h, c, w)
                    mem_rd = bass.AP(
                        tensor=mem_bt.tensor,
                        offset=mem_bt.offset,
                        ap=[mem_bt.ap[0], [3 * 512, NH], [1, 512], [512, 3]],
                    )
                    s_wr = bass.AP(
                        tensor=S.tensor,
                        offset=S.offset,
                        ap=[S.ap[0], [512 * 4, NH], [4, 512], [1, 3]],
                    )
                    nc.vector.tensor_scalar(
                        out=s_wr,
                        in0=mem_rd,
                        scalar1=THRESH,
                        scalar2=None,
                        op0=mybir.AluOpType.is_gt,
                    )
                    # plane 3 from (z, u3), both halves at once
                    s3_wr = bass.AP(
                        tensor=S.tensor,
                        offset=S.offset + 3,
                        ap=[S.ap[0], [512 * 4, NH], [4, 512]],
                    )
                    nc.vector._custom_dve(
                        LIF_SPIKE2, out=s3_wr, in0=t_z[bt], in1=t_u3[bt], s0=THRESH
                    )
                    nc.vector._custom_dve(
                        LIF_GATE2,
                        out=t_u3[bt],
                        in0=t_z[bt],
                        in1=t_u3[bt],
                        s0=THRESH,
                        s1=DECAY,
                    )

                    if t < T - 1:
                        upb_t = upbpool.tile(
                            [128, NH * 3, 512], F32, tag=f"upb{bt}", name=f"upb{bt}_{t}"
                        )
                        b_rd = bass.AP(
                            tensor=t_b.tensor,
                            offset=t_b.offset,
                            ap=[t_b.ap[0], [512, NH * 3], [1, 512]],
                        )
                        mem_rd6 = bass.AP(
                            tensor=mem_bt.tensor,
                            offset=mem_bt.offset,
                            ap=[mem_bt.ap[0], [512, NH * 3], [1, 512]],
                        )
                        nc.vector._custom_dve(
                            LIF_UPB,
                            out=upb_t,
                            in0=mem_rd6,
                            in1=b_rd,
                            s0=THRESH,
                            s1=DECAY,
                        )
                        upb_prev[bt] = upb_t

                        v = vpool.tile([128, NH, 3, 512], F32, tag="v")
                        for h in range(NH):
                            for w in range(3):
                                nc.scalar.activation(
                                    v[:, h, w, :],
                                    mem_bt[:, h, w, :],
                                    mybir.ActivationFunctionType.Identity,
                                    bias=t_wlb[:, 3:4] if w == 0 else 0.0,
                                    scale=t_wlb[:, w : w + 1],
                                )
                        zt = vpool.tile([128, NH, 512], F32, tag="ztmp")
                        for h in range(NH):
                            nc.gpsimd.tensor_tensor(
                                out=zt[:, h],
                                in0=v[:, h, 0, :],
                                in1=v[:, h, 1, :],
                                op=mybir.AluOpType.add,
                            )
                            nc.gpsimd.tensor_tensor(
                                out=t_z[bt][:, h],
                                in0=zt[:, h],
                                in1=v[:, h, 2, :],
                                op=mybir.AluOpType.add,
                            )

                    nc.sync.dma_start(
                        out=d_out[b0 : b0 + 128, t, :],
                        in_=S.rearrange("p c j -> p (c j)"),
                    )

    nc.finalize()
    return nc


_NC_CACHE = {}


def _get_nc(passes=3):
    if passes not in _NC_CACHE:
        _NC_CACHE[passes] = _build(passes)
    return _NC_CACHE[passes]


def kernel(**inputs):
    x = np.asarray(inputs["x"], dtype=np.float32)
    W = [np.asarray(inputs[f"W{i}"], dtype=np.float32) for i in (1, 2, 3)]
    bvec = [np.asarray(inputs[f"b{i}"], dtype=np.float32) for i in (1, 2, 3)]
    Wl = np.asarray(inputs["Wl"], dtype=np.float32)
    bl = np.asarray(inputs["bl"], dtype=np.float32)

    WT = np.stack([Wk.T for Wk in W], axis=1).astype(np.float32)  # [IN, 3, C]
    Wh = _round11(WT)
    Wl16 = (WT - Wh).astype(ml_dtypes.bfloat16)
    wh = np.ascontiguousarray(Wh.reshape(KC, 128, 3, C))
    wl16 = np.ascontiguousarray(Wl16.reshape(KC, 128, 3, C))
    b_cat = np.ascontiguousarray(np.stack(bvec, axis=0).reshape(1, 3, C))
    wlb = np.concatenate([Wl[0].reshape(3), bl.reshape(1)]).reshape(1, 4).astype(
        np.float32
    )

    xh = _round11(x)
    xl = _round11(x - xh)
    # per-core pre-transposed layout [NCORES, T, NBT, KC, 128(k), 128(b)]
    def to_T(a):
        # [B, T, IN] -> [cores, bloc, T, IN]
        a = a.reshape(NCORES, NBT, 128, T, KC, 128)
        return np.ascontiguousarray(a.transpose(0, 3, 1, 4, 5, 2))

    xhT = to_T(xh)
    xlT = to_T(xl)
    xh16T = xhT.astype(ml_dtypes.bfloat16)

    nc = _get_nc(3)
    in_maps = [
        dict(
            xhT=xhT[c],
            xh16T=xh16T[c],
            xlT=xlT[c],
            wh=wh,
            wl16=wl16,
            b=b_cat,
            wlb=wlb,
        )
        for c in range(NCORES)
    ]
    res = bass_utils.run_bass_kernel_spmd(nc, in_maps, core_ids=list(range(NCORES)))
    return np.concatenate([r["spk"] for r in res.results], axis=0)


if __name__ == "__main__":
    rng = np.random.default_rng(0)
    s_in = 1.0 / np.sqrt(IN)
    s3 = 1.0 / np.sqrt(3.0)
    ins = dict(
        x=rng.standard_normal((B, T, IN)).astype(np.float32),
        W1=rng.uniform(-s_in, s_in, (C, IN)).astype(np.float32),
        b1=rng.uniform(-s_in, s_in, (C,)).astype(np.float32),
        W2=rng.uniform(-s_in, s_in, (C, IN)).astype(np.float32),
        b2=rng.uniform(-s_in, s_in, (C,)).astype(np.float32),
        W3=rng.uniform(-s_in, s_in, (C, IN)).astype(np.float32),
        b3=rng.uniform(-s_in, s_in, (C,)).astype(np.float32),
        Wl=rng.uniform(-s3, s3, (1, 3)).astype(np.float32),
        bl=rng.uniform(-s3, s3, (1,)).astype(np.float32),
        wins=T,
    )
    out = kernel(**ins)

    # numpy reference
    p = [
        (ins["x"].reshape(B * T, IN) @ ins[f"W{k+1}"].T + ins[f"b{k+1}"]).reshape(
            B, T, C
        )
        for k in range(3)
    ]
    mem = np.zeros((B, C, 4), np.float32)
    spk = np.zeros((B, C, 4), np.float32)
    exp = np.zeros((B, T, C, 4), np.float32)
    for t in range(T):
        inner = mem[..., :3] @ ins["Wl"][0] + ins["bl"][0]
        ia = np.stack([p[0][:, t], p[1][:, t], p[2][:, t], inner], axis=-1)
        mem = mem * np.float32(0.2) * (1.0 - spk) + ia
        spk = (mem > 0.8).astype(np.float32)
        exp[:, t] = spk
    exp = exp.reshape(B, T, C * 4)
    rel = np.linalg.norm(out - exp) / np.linalg.norm(exp)
    print("out", out.shape, out.dtype, "density", out.mean())
    print("rel err vs numpy fp32:", rel, "nflips", np.abs(out - exp).sum())



# revision 44
# speedup vs baseline: 2.1367x; 2.1367x over previous
"""Trainium2 Bass kernel for nn_LIF_hh_neuron (B=2048, T=15, IN=512, C=1024).

Sharding: pure data-parallel over batch B across 8 NeuronCores (256 each).

Active design (scheme "B", ~282 us modeled vs 602 us baseline measured):
  - Projections p_k(t) = x_t @ W_k.T run as ONE full-rate f32r pass
    (xh = round11(x) @ Wh = round11(W)) plus ONE fp8e5 DoubleRow
    correction matmul at half rate covering both rounding residuals:
    [e*16; xh/64] @ [W/16; (W-Wh)*64] with e = x - xh (contraction 1024,
    0.5 cycles/row).  End-to-end spike rel err ~0.004 (gate is 2e-2).
  - Matmuls run in quarter sub-blocks (256 psum cols, 4KB bank-aligned
    psum tiles, 4-deep rotation) so the drain->preload bank recycle never
    stalls the PE; the previous block's spike/gate/S-DMA and v/z updates
    are emitted inside the next block's quarter loop (SG_Q/VZ_Q knobs) to
    keep the in-order DVE/ACT queues from bunching.  Bias b is preloaded into PSUM by the ACT engine and
    all matmuls accumulate (start=False); the first-ever use of each bank
    (t=0, bt=0) instead uses start=True to clear inherited PSUM state and
    receives b through the fused drain (prev-mem primed with b/DECAY).
  - Per timestep, per batch-tile (128 rows), all state stays on-chip in a
    w-last mem4 [128, NH, 512, 4] tile (planes 0-2 = matmul memories,
    plane 3 = recurrent inner-product memory):
      mem'(t)  = psum + select(pm<=0.8, 0.2*pm, 0)   [fused LIF_DRAIN, DVE]
      spike(t) = mem4 > 0.8 -> uint8 S               [one DVE tensor_scalar]
      u3'      = select(m3<=0.8, 0.2*m3, 0)          [custom DVE gate]
      v_w      = wl_w*mem_w (+bl for w=0)            [ACT scaled copies]
      z(t+1)   = v0+v1+v2, m3(t+1) = z+u3            [GPSIMD adds]
    Spikes stream out as uint8 and are widened to f32 on the host (exact).
  - Engine balance per block: PE 7.7us, DVE ~7.2us, ACT ~6.9us, Pool
    ~6.4us; DMA (u8 out) well under.

Scheme "A" (fallback, ~342 us modeled, rel err ~0.013): f32r pass + bf16
residual pass (xh@Wh + bf16(xh)@bf16(W-Wh)), upb custom op instead of the
fused drain, no psum preload.
"""

import numpy as np
import ml_dtypes

import concourse.bass as bass
import concourse.mybir as mybir
import concourse.tile as tile
from concourse import bacc
from concourse import bass_utils
from concourse.dve_spec import Spec, Src0, Src1, C0, C1, Zero, select, lower
from concourse.dve_ops import has_src1, DveOp, OPS
import concourse.dve_ops as dve_ops_mod
from concourse.dve_uop import DveOpSpec

F32 = mybir.dt.float32
F32R = mybir.dt.float32r
BF16 = mybir.dt.bfloat16

B, T, IN, C = 2048, 15, 512, 1024
NCORES = 8
BLOC = B // NCORES          # 256 batches per core
NBT = BLOC // 128           # 2 batch tiles per core
KC = IN // 128              # 4 contraction chunks
NH = 2                      # two c-halves of 512
THRESH = 0.8
DECAY = 0.2


def _register_op(name, spec, subdim=False):
    for existing in OPS:
        if existing.name == name:
            return existing
    op = DveOp(name, spec, subdim=subdim, uops_sha={})
    OPS.append(op)
    dve_ops_mod._SUB_OPCODE_FOR_NAME[name] = (
        dve_ops_mod._CUSTOM_DVE_ROW_BASE + len(OPS) - 1
    )
    dve_ops_mod.CUSTOM_DVE_SPECS[name] = spec
    shas = {}
    for ver in ("v3", "v4"):
        s = DveOpSpec(
            name=name,
            opcode=dve_ops_mod.get_dve_sub_opcode(name),
            uops=lower(spec, ver=ver),
            rd1_en=has_src1(spec),
        )
        shas[ver] = s.sha(ver)
    object.__setattr__(op, "uops_sha", shas)
    return op


# upb = select(mem <= thr, mem*decay, 0) + b
LIF_UPB = _register_op(
    "LIF_UPB_ANT",
    Spec(
        body=select(Src0 <= C0, Src0 * C1, Zero) + Src1,
        reference=lambda in0, in1, s0, s1, *a: (
            np.where(in0 <= s0, in0 * s1, 0.0) + in1.reshape(in0.shape)
        ).astype(np.float32),
    ),
)

# s3 = (z + u3) > thr
LIF_SPIKE2 = _register_op(
    "LIF_SPIKE2_ANT",
    Spec(
        body=(Src0 + Src1) > C0,
        reference=lambda in0, in1, s0, s1, *a: (
            (in0 + in1.reshape(in0.shape)) > s0
        ).astype(np.float32),
    ),
)

# u3' = select(z + u3 <= thr, (z + u3)*decay, 0)
LIF_GATE2 = _register_op(
    "LIF_GATE2_ANT",
    Spec(
        body=select((Src0 + Src1) <= C0, (Src0 + Src1) * C1, Zero),
        reference=lambda in0, in1, s0, s1, *a: np.where(
            (in0 + in1.reshape(in0.shape)) <= s0,
            (in0 + in1.reshape(in0.shape)) * s1,
            0.0,
        ).astype(np.float32),
    ),
)

# u3' = select(m3 <= thr, m3*decay, 0)   [1-src gate from mem4 plane 3]
LIF_GATE1 = _register_op(
    "LIF_GATE1_ANT",
    Spec(
        body=select(Src0 <= C0, Src0 * C1, Zero),
        reference=lambda in0, in1, s0, s1, *a: np.where(
            in0 <= s0, in0 * s1, 0.0
        ).astype(np.float32),
    ),
)

# mem' = ps + select(pm <= thr, pm*decay, 0)   [fused drain, b preloaded in ps]
LIF_DRAIN = _register_op(
    "LIF_DRAIN_ANT",
    Spec(
        body=Src0 + select(Src1 <= C0, Src1 * C1, Zero),
        reference=lambda in0, in1, s0, s1, *a: (
            in0 + np.where(
                in1.reshape(in0.shape) <= s0, in1.reshape(in0.shape) * s1, 0.0
            )
        ).astype(np.float32),
    ),
)


def _round11(a):
    """Round fp32 mantissa to 11 explicit bits (the f32r grid), nearest-even."""
    u = np.ascontiguousarray(a, np.float32).view(np.uint32)
    half = np.uint32(1 << 11)
    mask = np.uint32((1 << 12) - 1)
    frac = u & mask
    u2 = u & ~mask
    rup = (frac > half) | (
        (frac == half) & ((u2 >> np.uint32(12)) & np.uint32(1)).astype(bool)
    )
    return (u2 + np.where(rup, np.uint32(1 << 12), np.uint32(0))).view(np.float32)


def _build(passes=3, reps=1):
    nc = bacc.Bacc("TRN2", target_bir_lowering=False, debug=False)

    # pre-transposed, pre-split x: [T, NBT, KC, 128(k), 128(b)]
    d_xhT = nc.dram_tensor("xhT", [T, NBT, KC, 128, 128], F32, kind="ExternalInput").ap()
    d_xh16T = nc.dram_tensor(
        "xh16T", [T, NBT, KC, 128, 128], BF16, kind="ExternalInput"
    ).ap()
    d_xlT = None
    if passes == 3:
        d_xlT = nc.dram_tensor(
            "xlT", [T, NBT, KC, 128, 128], F32, kind="ExternalInput"
        ).ap()
    d_wh = nc.dram_tensor("wh", [KC, 128, 3, C], F32, kind="ExternalInput").ap()
    d_wl16 = nc.dram_tensor("wl16", [KC, 128, 3, C], BF16, kind="ExternalInput").ap()
    d_b = nc.dram_tensor("b", [1, 3, C], F32, kind="ExternalInput").ap()
    d_wlb = nc.dram_tensor("wlb", [1, 4], F32, kind="ExternalInput").ap()
    d_out = nc.dram_tensor("spk", [BLOC, T, 4 * C], F32, kind="ExternalOutput").ap()

    with tile.TileContext(nc) as tc:
        with (
            tc.tile_pool(name="wpool", bufs=1) as wpool,
            tc.tile_pool(name="state", bufs=1) as state,
            tc.tile_pool(name="mem", bufs=1) as mempool,
            tc.tile_pool(name="upb", bufs=1) as upbpool,
            tc.tile_pool(name="vpool", bufs=1) as vpool,
            tc.tile_pool(name="spool", bufs=SPOOL_BUFS) as spool,
            tc.tile_pool(name="xin", bufs=XIN_BUFS) as xin,
            tc.tile_pool(name="pspool", bufs=2, space="PSUM") as pspool,
        ):
            # ---- static tiles ----
            t_wh = wpool.tile([128, KC, 3, C], F32R, tag="wh")
            nc.sync.dma_start(
                out=t_wh, in_=d_wh.rearrange("k p w c -> p k w c").bitcast(F32R)
            )
            t_wl16 = wpool.tile([128, KC, 3, C], BF16, tag="wl16")
            nc.sync.dma_start(out=t_wl16, in_=d_wl16.rearrange("k p w c -> p k w c"))
            t_b = wpool.tile([128, NH, 3, 512], F32, tag="b")
            for h in range(NH):
                nc.sync.dma_start(
                    out=t_b[:, h],
                    in_=bass.AP(
                        tensor=d_b.tensor,
                        offset=h * 512,
                        ap=[[0, 128], [C, 3], [1, 512]],
                    ),
                )
            t_wlb = wpool.tile([128, 4], F32, tag="wlb")
            nc.sync.dma_start(
                out=t_wlb,
                in_=bass.AP(tensor=d_wlb.tensor, offset=0, ap=[[0, 128], [1, 4]]),
            )

            # ---- per-bt recurrent state ----
            t_z = [
                state.tile([128, NH, 512], F32, tag=f"z{bt}", name=f"z{bt}")
                for bt in range(NBT)
            ]
            t_u3 = [
                state.tile([128, NH, 512], F32, tag=f"u3{bt}", name=f"u3{bt}")
                for bt in range(NBT)
            ]

            def emit(rep):
                for bt in range(NBT):
                    nc.vector.memset(t_u3[bt], 0.0)
                    nc.scalar.activation(
                        t_z[bt],
                        t_u3[bt],
                        mybir.ActivationFunctionType.Identity,
                        bias=t_wlb[:, 3:4],
                        scale=1.0,
                    )

                upb_prev = [None] * NBT  # per-bt [128, 2*3, 512]; None -> b at t=0

                for t in range(T):
                    for bt in range(NBT):
                        b0 = bt * 128
                        xhT = xin.tile([128, KC, 128], F32R, tag="xhT")
                        nc.sync.dma_start(
                            out=xhT,
                            in_=d_xhT[t, bt].rearrange("k p b -> p k b").bitcast(F32R),
                        )
                        xhT16 = xin.tile([128, KC, 128], BF16, tag="xhT16")
                        nc.sync.dma_start(
                            out=xhT16, in_=d_xh16T[t, bt].rearrange("k p b -> p k b")
                        )
                        plist = [(xhT, t_wh), (xhT16, t_wl16)]
                        if passes == 3:
                            xlT = xin.tile([128, KC, 128], F32R, tag="xlT")
                            nc.sync.dma_start(
                                out=xlT,
                                in_=d_xlT[t, bt]
                                .rearrange("k p b -> p k b")
                                .bitcast(F32R),
                            )
                            plist.append((xlT, t_wh))
                        elif passes == 1:
                            plist = plist[:1]
                        np_ = len(plist)

                        ps = [None] * NH
                        for h in range(NH):
                            ps[h] = pspool.tile(
                                [128, 3, 512], F32, tag="ps", name=f"ps_{rep}_{t}_{bt}_{h}"
                            )
                        for k in range(KC):
                            for pi, (lhs, w_t) in enumerate(plist):
                                for h in range(NH):
                                    c0 = h * 512
                                    for w in range(3):
                                        nc.tensor.matmul(
                                            ps[h][:, w, :],
                                            lhs[:, k, :],
                                            w_t[:, k, w, c0 : c0 + 512],
                                            start=(k == 0 and pi == 0),
                                            stop=(k == KC - 1 and pi == np_ - 1),
                                        )

                        # mem(t) = psum + upb(t)   [per half]
                        mem_bt = mempool.tile(
                            [128, NH, 3, 512], F32, tag=f"mem{bt}", name=f"mem{bt}_{rep}_{t}"
                        )
                        ub = upb_prev[bt]
                        for h in range(NH):
                            if ub is None:
                                ub_h = t_b[:, h]
                            else:
                                ub_h = ub[:, h * 3 : (h + 1) * 3, :]
                            nc.vector.tensor_tensor(
                                out=mem_bt[:, h],
                                in0=ps[h],
                                in1=ub_h,
                                op=mybir.AluOpType.add,
                            )

                        # spikes for all 4 planes, interleaved (c, j) in one tile
                        S = spool.tile([128, 2 * 512, 4], F32, tag="S")
                        # planes 0-2: read-strided over (h, c, w)
                        mem_rd = bass.AP(
                            tensor=mem_bt.tensor,
                            offset=mem_bt.offset,
                            ap=[mem_bt.ap[0], [3 * 512, NH], [1, 512], [512, 3]],
                        )
                        s_wr = bass.AP(
                            tensor=S.tensor,
                            offset=S.offset,
                            ap=[S.ap[0], [512 * 4, NH], [4, 512], [1, 3]],
                        )
                        nc.vector.tensor_scalar(
                            out=s_wr,
                            in0=mem_rd,
                            scalar1=THRESH,
                            scalar2=None,
                            op0=mybir.AluOpType.is_gt,
                        )
                        # plane 3 from (z, u3), both halves at once
                        s3_wr = bass.AP(
                            tensor=S.tensor,
                            offset=S.offset + 3,
                            ap=[S.ap[0], [512 * 4, NH], [4, 512]],
                        )
                        nc.vector._custom_dve(
                            LIF_SPIKE2, out=s3_wr, in0=t_z[bt], in1=t_u3[bt], s0=THRESH
                        )
                        nc.vector._custom_dve(
                            LIF_GATE2,
                            out=t_u3[bt],
                            in0=t_z[bt],
                            in1=t_u3[bt],
                            s0=THRESH,
                            s1=DECAY,
                        )

                        if t < T - 1:
                            upb_t = upbpool.tile(
                                [128, NH * 3, 512],
                                F32,
                                tag=f"upb{bt}",
                                name=f"upb{bt}_{rep}_{t}",
                            )
                            b_rd = bass.AP(
                                tensor=t_b.tensor,
                                offset=t_b.offset,
                                ap=[t_b.ap[0], [512, NH * 3], [1, 512]],
                            )
                            mem_rd6 = bass.AP(
                                tensor=mem_bt.tensor,
                                offset=mem_bt.offset,
                                ap=[mem_bt.ap[0], [512, NH * 3], [1, 512]],
                            )
                            nc.vector._custom_dve(
                                LIF_UPB,
                                out=upb_t,
                                in0=mem_rd6,
                                in1=b_rd,
                                s0=THRESH,
                                s1=DECAY,
                            )
                            upb_prev[bt] = upb_t

                            v = vpool.tile([128, NH, 3, 512], F32, tag="v")
                            for h in range(NH):
                                for w in range(3):
                                    nc.scalar.activation(
                                        v[:, h, w, :],
                                        mem_bt[:, h, w, :],
                                        mybir.ActivationFunctionType.Identity,
                                        bias=t_wlb[:, 3:4] if w == 0 else 0.0,
                                        scale=t_wlb[:, w : w + 1],
                                    )
                            zt = vpool.tile([128, NH, 512], F32, tag="ztmp")
                            for h in range(NH):
                                nc.gpsimd.tensor_tensor(
                                    out=zt[:, h],
                                    in0=v[:, h, 0, :],
                                    in1=v[:, h, 1, :],
                                    op=mybir.AluOpType.add,
                                )
                                nc.gpsimd.tensor_tensor(
                                    out=t_z[bt][:, h],
                                    in0=zt[:, h],
                                    in1=v[:, h, 2, :],
                                    op=mybir.AluOpType.add,
                                )

                        nc.sync.dma_start(
                            out=d_out[b0 : b0 + 128, t, :],
                            in_=S.rearrange("p c j -> p (c j)"),
                        )

            for rep in range(reps):
                emit(rep)

    nc.finalize()
    return nc


FP8 = mybir.dt.float8e5

# fp8 correction pass scales: (e*SE)@(W/SE) + (xh/SX)@(F*SX)
SE = 16.0     # 2**4
SX = 64.0     # 2**6
KCP = 4       # DoubleRow k-pair chunks for 1024-contraction correction


FLUSH_DELAYED = True
XIN_BUFS = 2
SPOOL_BUFS = 2
SG_Q = 0     # quarter index at which prev-block spike/gate/S-DMA are emitted
VZ_Q = 0
VZ_Q2 = 1
VZ_N = 1     # quarter index for prev-block v/z (99 = end of block)


def _build2(scheme="A", reps=1):
    """Restructured kernel.

    scheme "A": 2-pass matmul (xh f32r @ Wh + xh bf16 @ Wl16), upb custom op.
    scheme "B": f32r pass + fp8e5 DoubleRow correction pass; bias preloaded
                into PSUM (ACT); fused drain custom op (no upb tile).

    Matmul runs in quarter sub-blocks (256 output cols) with a 4-deep PSUM
    rotation so the drain->preload bank recycle never stalls the PE.
    Common: mem4 [128, NH, 512, 4] w-last layout; plane 3 = z + u3 written by
    Pool; one is_gt over all 4 planes -> u8 S tile; u8 output DMA.
    """
    nc = bacc.Bacc("TRN2", target_bir_lowering=False, debug=False)
    NQ = 4          # quarter sub-blocks per (t, bt)
    QC = C // NQ    # 256 psum columns per quarter

    d_xhT = nc.dram_tensor("xhT", [T, NBT, KC, 128, 128], F32, kind="ExternalInput").ap()
    if scheme == "A":
        d_x2 = nc.dram_tensor(
            "xh16T", [T, NBT, KC, 128, 128], BF16, kind="ExternalInput"
        ).ap()
        d_w2 = nc.dram_tensor("wl16", [KC, 128, 3, C], BF16, kind="ExternalInput").ap()
    else:
        d_x2 = nc.dram_tensor(
            "xcorr", [T, NBT, KCP, 128, 2, 128], FP8, kind="ExternalInput"
        ).ap()
        d_w2 = nc.dram_tensor(
            "wcorr", [KCP, 128, 2, 3, C], FP8, kind="ExternalInput"
        ).ap()
    d_wh = nc.dram_tensor("wh", [KC, 128, 3, C], F32, kind="ExternalInput").ap()
    d_b = nc.dram_tensor("b_wc", [1, NH, 3, 512], F32, kind="ExternalInput").ap()
    d_wlb = nc.dram_tensor("wlb", [1, 4], F32, kind="ExternalInput").ap()
    d_out = nc.dram_tensor(
        "spk", [BLOC, T, 4 * C], mybir.dt.uint8, kind="ExternalOutput"
    ).ap()

    with tile.TileContext(nc) as tc:
        with (
            tc.tile_pool(name="wpool", bufs=1) as wpool,
            tc.tile_pool(name="state", bufs=1) as state,
            tc.tile_pool(name="mem", bufs=(2 if scheme == "B" else 1)) as mempool,
            tc.tile_pool(name="upb", bufs=1) as upbpool,
            tc.tile_pool(name="vpool", bufs=1) as vpool,
            tc.tile_pool(name="spool", bufs=SPOOL_BUFS) as spool,
            tc.tile_pool(name="xin", bufs=XIN_BUFS) as xin,
            tc.tile_pool(name="pspool", bufs=4, space="PSUM") as pspool,
        ):
            # ---- static tiles ----
            # small tensors first (b gates the psum preloads), then the big
            # weight streams chunk-by-chunk so block-0 matmuls can start as
            # soon as their chunk lands.
            t_b = wpool.tile([128, NH, 3, 512], F32, tag="b")
            nc.sync.dma_start(
                out=t_b,
                in_=bass.AP(
                    tensor=d_b.tensor,
                    offset=0,
                    ap=[[0, 128], [3 * 512, NH], [512, 3], [1, 512]],
                ),
            )
            t_wlb = wpool.tile([128, 4], F32, tag="wlb")
            nc.sync.dma_start(
                out=t_wlb,
                in_=bass.AP(tensor=d_wlb.tensor, offset=0, ap=[[0, 128], [1, 4]]),
            )
            # prefetch the first block's x tiles ahead of the weight stream
            prefetched_x = {}

            def load_x(t, bt):
                xhT = xin.tile([128, KC, 128], F32R, tag="xhT")
                nc.sync.dma_start(
                    out=xhT,
                    in_=d_xhT[t, bt].rearrange("k p b -> p k b").bitcast(F32R),
                )
                if scheme == "A":
                    x2 = xin.tile([128, KC, 128], BF16, tag="x2")
                    nc.sync.dma_start(
                        out=x2, in_=d_x2[t, bt].rearrange("k p b -> p k b")
                    )
                else:
                    x2 = xin.tile([128, KCP, 2, 128], FP8, tag="x2")
                    nc.sync.dma_start(
                        out=x2, in_=d_x2[t, bt].rearrange("k p i b -> p k i b")
                    )
                return (xhT, x2)

            prefetched_x[(0, 0)] = load_x(0, 0)

            t_wh_k = []
            for k in range(KC):
                tk = wpool.tile([128, 3, C], F32R, tag=f"wh{k}")
                nc.sync.dma_start(out=tk, in_=d_wh[k].bitcast(F32R))
                t_wh_k.append(tk)
            t_w2_k = []
            if scheme == "A":
                for k in range(KC):
                    tk = wpool.tile([128, 3, C], BF16, tag=f"w2{k}")
                    nc.sync.dma_start(out=tk, in_=d_w2[k])
                    t_w2_k.append(tk)
            else:
                for k in range(KCP):
                    tk = wpool.tile([128, 2, 3, C], FP8, tag=f"w2{k}")
                    nc.sync.dma_start(out=tk, in_=d_w2[k])
                    t_w2_k.append(tk)

            # ---- per-bt recurrent state ----
            t_z = [
                state.tile([128, NH, 512], F32, tag=f"z{bt}", name=f"z{bt}")
                for bt in range(NBT)
            ]
            t_u3 = [
                state.tile([128, NH, 512], F32, tag=f"u3{bt}", name=f"u3{bt}")
                for bt in range(NBT)
            ]

            # ---- psum quarter tiles: alloc + bias-preload in-loop ----
            ps_count = [0]

            def alloc_ps(q, first_use):
                # Pad to 4KB (2 banks) so start=True zero-regions (2KB
                # granularity) stay within this tile: w0 start zeroes the
                # w0+w1 bank, w2 start zeroes the w2+pad bank.
                n = ps_count[0]
                ps_count[0] += 1
                h, c0 = q // 2, (q % 2) * QC
                ps = pspool.tile([128, 4, QC], F32, tag="ps", name=f"ps_{n}")
                if scheme == "B" and not first_use:
                    # bias preload; first-use tiles (t=0, bt=0) are cleared
                    # by start=True instead and get b via the drain's pm.
                    nc.scalar.activation(
                        ps[:, 0:3, :],
                        bass.AP(
                            tensor=t_b.tensor,
                            offset=t_b.offset + h * 3 * 512 + c0,
                            ap=[t_b.ap[0], [512, 3], [1, QC]],
                        ),
                        mybir.ActivationFunctionType.Identity,
                        bias=0.0,
                        scale=1.0,
                    )
                return ps

            def emit(rep):
                for bt in range(NBT):
                    nc.vector.memset(t_u3[bt], 0.0)
                    nc.scalar.activation(
                        t_z[bt],
                        t_u3[bt],
                        mybir.ActivationFunctionType.Identity,
                        bias=t_wlb[:, 3:4],
                        scale=1.0,
                    )

                mem_prev = [None] * NBT   # B: previous mem4 tile
                upb_prev = [None] * NBT   # A: upb tile (None -> b at t=0)
                pending_vz = []           # delayed (mem4, bt, t) for v/z

                def flush_vz(nmax=99):
                    n = 0
                    while pending_vz and n < nmax:
                        m4p, btp, tp, hp = pending_vz.pop(0)
                        n += 1
                        if tp < T - 1:
                            emit_vz(m4p, btp, hp)
                pending_sg = []           # delayed (mem4, bt, t) spike+gate

                def flush_sg():
                    while pending_sg:
                        m4, bt_, t_ = pending_sg.pop(0)
                        S = spool.tile([128, 4 * C], mybir.dt.uint8, tag="S")
                        nc.vector.tensor_scalar(
                            out=S,
                            in0=m4.rearrange("p h c j -> p (h c j)"),
                            scalar1=THRESH,
                            scalar2=None,
                            op0=mybir.AluOpType.is_gt,
                        )
                        m3_rd = bass.AP(
                            tensor=m4.tensor,
                            offset=m4.offset + 3,
                            ap=[m4.ap[0], [512 * 4, NH], [4, 512]],
                        )
                        if t_ < T - 1:
                            nc.vector._custom_dve(
                                LIF_GATE1,
                                out=t_u3[bt_],
                                in0=m3_rd,
                                s0=THRESH,
                                s1=DECAY,
                            )
                        nc.sync.dma_start(
                            out=d_out[bt_ * 128 : bt_ * 128 + 128, t_, :],
                            in_=S,
                        )

                def emit_vz(mem4_, bt_, h):
                    v = vpool.tile([128, 3, 512], F32, tag=f"v{h}")
                    for w in range(3):
                        nc.scalar.activation(
                            v[:, w, :],
                            bass.AP(
                                tensor=mem4_.tensor,
                                offset=mem4_.offset + h * 512 * 4 + w,
                                ap=[mem4_.ap[0], [4, 512]],
                            ),
                            mybir.ActivationFunctionType.Identity,
                            bias=t_wlb[:, 3:4] if w == 0 else 0.0,
                            scale=t_wlb[:, w : w + 1],
                        )
                    zt = vpool.tile([128, 512], F32, tag=f"ztmp{h}")
                    nc.gpsimd.tensor_tensor(
                        out=zt,
                        in0=v[:, 0, :],
                        in1=v[:, 1, :],
                        op=mybir.AluOpType.add,
                    )
                    nc.gpsimd.tensor_tensor(
                        out=t_z[bt_][:, h],
                        in0=zt,
                        in1=v[:, 2, :],
                        op=mybir.AluOpType.add,
                    )

                if scheme == "B":
                    for bt in range(NBT):
                        mz = mempool.tile(
                            [128, NH, 512, 4], F32, tag=f"mem{bt}",
                            name=f"memz{bt}_{rep}",
                        )
                        nc.vector.memset(mz, 0.0)
                        if rep == 0 and bt == 0:
                            # planes 0-2 <- b/DECAY: the t=0 drain's gated
                            # decay maps it back to +b (|5b| <= thr always),
                            # replacing the psum preload zeroed by start=True.
                            for h in range(NH):
                                nc.scalar.activation(
                                    bass.AP(
                                        tensor=mz.tensor,
                                        offset=mz.offset + h * 512 * 4,
                                        ap=[mz.ap[0], [1, 3], [4, 512]],
                                    ),
                                    t_b[:, h],
                                    mybir.ActivationFunctionType.Identity,
                                    bias=0.0,
                                    scale=1.0 / DECAY,
                                )
                        mem_prev[bt] = mz

                for t in range(T):
                    for bt in range(NBT):
                        b0 = bt * 128
                        if (t, bt) in prefetched_x and rep == 0:
                            xhT, x2 = prefetched_x.pop((t, bt))
                        else:
                            xhT, x2 = load_x(t, bt)

                        if scheme == "B":
                            mem4 = mempool.tile(
                                [128, NH, 512, 4], F32, tag=f"mem{bt}",
                                name=f"mem{bt}_{rep}_{t}",
                            )
                        else:
                            mem4 = mem_prev[bt] if mem_prev[bt] is not None else (
                                mempool.tile(
                                    [128, NH, 512, 4], F32, tag=f"mem{bt}",
                                    name=f"mem{bt}_{rep}",
                                )
                            )
                            mem_prev[bt] = mem4

                        # plane 3 <- z + u3 (Pool), per h
                        for h in range(NH):
                            nc.gpsimd.tensor_tensor(
                                out=bass.AP(
                                    tensor=mem4.tensor,
                                    offset=mem4.offset + h * 512 * 4 + 3,
                                    ap=[mem4.ap[0], [4, 512]],
                                ),
                                in0=t_z[bt][:, h],
                                in1=t_u3[bt][:, h],
                                op=mybir.AluOpType.add,
                            )

                        for q in range(NQ):
                            h, c0 = q // 2, (q % 2) * QC
                            cg = h * 512 + c0  # global col offset
                            ps = alloc_ps(
                                q, rep == 0 and t == 0 and bt == 0
                            )
                            if scheme == "B":
                                first_use = rep == 0 and t == 0 and bt == 0
                                for k in range(KC):
                                    for w in range(3):
                                        nc.tensor.matmul(
                                            ps[:, w, :],
                                            xhT[:, k, :],
                                            t_wh_k[k][:, w, cg : cg + QC],
                                            start=(
                                                first_use and k == 0
                                                and w in (0, 2)
                                            ),
                                            stop=False,
                                            skip_group_check=True,
                                        )
                                for k in range(KCP):
                                    for w in range(3):
                                        nc.tensor.matmul(
                                            ps[:, w, :],
                                            x2[:, k],
                                            t_w2_k[k][:, :, w, cg : cg + QC],
                                            start=False,
                                            stop=(k == KCP - 1),
                                            skip_group_check=True,
                                            perf_mode=mybir.MatmulPerfMode.DoubleRow,
                                        )
                            else:
                                plist = [(xhT, t_wh_k), (x2, t_w2_k)]
                                for pi, (lhs, w_t) in enumerate(plist):
                                    for k in range(KC):
                                        for w in range(3):
                                            nc.tensor.matmul(
                                                ps[:, w, :],
                                                lhs[:, k, :],
                                                w_t[k][:, w, cg : cg + QC],
                                                start=(
                                                    pi == 0 and k == 0
                                                    and w in (0, 2)
                                                ),
                                                stop=(pi == 1 and k == KC - 1),
                                                skip_group_check=True,
                                            )

                            # drain psum -> mem4 planes 0-2, (w,c) order
                            mem_wr = bass.AP(
                                tensor=mem4.tensor,
                                offset=mem4.offset + h * 512 * 4 + c0 * 4,
                                ap=[mem4.ap[0], [1, 3], [4, QC]],
                            )
                            if scheme == "B":
                                pm_rd = bass.AP(
                                    tensor=mem_prev[bt].tensor,
                                    offset=mem_prev[bt].offset + h * 512 * 4 + c0 * 4,
                                    ap=[mem_prev[bt].ap[0], [1, 3], [4, QC]],
                                )
                                nc.vector._custom_dve(
                                    LIF_DRAIN,
                                    out=mem_wr,
                                    in0=ps[:, 0:3, :],
                                    in1=pm_rd,
                                    s0=THRESH,
                                    s1=DECAY,
                                )
                            else:
                                ub = upb_prev[bt]
                                if ub is None:
                                    ub_h = bass.AP(
                                        tensor=t_b.tensor,
                                        offset=t_b.offset + h * 3 * 512 + c0,
                                        ap=[t_b.ap[0], [512, 3], [1, QC]],
                                    )
                                else:
                                    ub_h = bass.AP(
                                        tensor=ub.tensor,
                                        offset=ub.offset + h * 3 * 512 + c0,
                                        ap=[ub.ap[0], [512, 3], [1, QC]],
                                    )
                                nc.vector.tensor_tensor(
                                    out=mem_wr,
                                    in0=ps[:, 0:3, :],
                                    in1=ub_h,
                                    op=mybir.AluOpType.add,
                                )

                            if q == SG_Q and scheme == "B" and FLUSH_DELAYED:
                                flush_sg()
                            if q == VZ_Q and scheme == "B":
                                flush_vz(VZ_N)
                            if q == VZ_Q2 and scheme == "B":
                                flush_vz()

                        if scheme == "B":
                            mem_prev[bt] = mem4
                        pending_sg.append((mem4, bt, t))
                        if scheme == "A" or not FLUSH_DELAYED:
                            flush_sg()

                        if scheme == "B":
                            if VZ_Q >= NQ:
                                flush_vz()
                            pending_vz.append((mem4, bt, t, 0))
                            pending_vz.append((mem4, bt, t, 1))
                        elif t < T - 1:
                            upb_t = upbpool.tile(
                                [128, NH, 3, 512], F32, tag=f"upb{bt}",
                                name=f"upb{bt}_{rep}_{t}",
                            )
                            for h in range(NH):
                                mem_rd = bass.AP(
                                    tensor=mem4.tensor,
                                    offset=mem4.offset + h * 512 * 4,
                                    ap=[mem4.ap[0], [1, 3], [4, 512]],
                                )
                                nc.vector._custom_dve(
                                    LIF_UPB,
                                    out=upb_t[:, h],
                                    in0=mem_rd,
                                    in1=t_b[:, h],
                                    s0=THRESH,
                                    s1=DECAY,
                                )
                            upb_prev[bt] = upb_t
                            emit_vz(mem4, bt, 0)
                            emit_vz(mem4, bt, 1)

                flush_sg()

            for rep in range(reps):
                emit(rep)

    nc.finalize()
    return nc


_NC_CACHE = {}


def _get_nc(passes=3, reps=1):
    key = (passes, reps)
    if key not in _NC_CACHE:
        _NC_CACHE[key] = _build(passes, reps)
    return _NC_CACHE[key]


def _prepare_in_maps(inputs, passes=3):
    x = np.asarray(inputs["x"], dtype=np.float32)
    W = [np.asarray(inputs[f"W{i}"], dtype=np.float32) for i in (1, 2, 3)]
    bvec = [np.asarray(inputs[f"b{i}"], dtype=np.float32) for i in (1, 2, 3)]
    Wl = np.asarray(inputs["Wl"], dtype=np.float32)
    bl = np.asarray(inputs["bl"], dtype=np.float32)

    WT = np.stack([Wk.T for Wk in W], axis=1).astype(np.float32)  # [IN, 3, C]
    Wh = _round11(WT)
    Wl16 = (WT - Wh).astype(ml_dtypes.bfloat16)
    wh = np.ascontiguousarray(Wh.reshape(KC, 128, 3, C))
    wl16 = np.ascontiguousarray(Wl16.reshape(KC, 128, 3, C))
    b_cat = np.ascontiguousarray(np.stack(bvec, axis=0).reshape(1, 3, C))
    wlb = np.concatenate([Wl[0].reshape(3), bl.reshape(1)]).reshape(1, 4).astype(
        np.float32
    )

    xh = _round11(x)
    # per-core pre-transposed layout [NCORES, T, NBT, KC, 128(k), 128(b)]
    def to_T(a):
        # [B, T, IN] -> [cores, bloc, T, IN]
        a = a.reshape(NCORES, NBT, 128, T, KC, 128)
        return np.ascontiguousarray(a.transpose(0, 3, 1, 4, 5, 2))

    xhT = to_T(xh)
    xh16T = xhT.astype(ml_dtypes.bfloat16)
    maps = [
        dict(
            xhT=xhT[c],
            xh16T=xh16T[c],
            wh=wh,
            wl16=wl16,
            b=b_cat,
            wlb=wlb,
        )
        for c in range(NCORES)
    ]
    if passes == 3:
        xl = _round11(x - xh)
        xlT = to_T(xl)
        for c in range(NCORES):
            maps[c]["xlT"] = xlT[c]
    return maps


def _get_nc2(scheme="A", reps=1):
    key = ("v2", scheme, reps)
    if key not in _NC_CACHE:
        _NC_CACHE[key] = _build2(scheme, reps)
    return _NC_CACHE[key]


def _prepare_in_maps2(inputs, scheme="A"):
    x = np.asarray(inputs["x"], dtype=np.float32)
    W = [np.asarray(inputs[f"W{i}"], dtype=np.float32) for i in (1, 2, 3)]
    bvec = [np.asarray(inputs[f"b{i}"], dtype=np.float32) for i in (1, 2, 3)]
    Wl = np.asarray(inputs["Wl"], dtype=np.float32)
    bl = np.asarray(inputs["bl"], dtype=np.float32)

    WT = np.stack([Wk.T for Wk in W], axis=1).astype(np.float32)  # [IN, 3, C]
    Wh = _round11(WT)
    wh = np.ascontiguousarray(Wh.reshape(KC, 128, 3, C))
    # b in per-h (w,c) layout [1, NH, 3, 512]
    b_cat = np.stack(bvec, axis=0)  # [3, C]
    b_wc = np.ascontiguousarray(
        b_cat.reshape(3, NH, 512).transpose(1, 0, 2).reshape(1, NH, 3, 512)
    )
    wlb = np.concatenate([Wl[0].reshape(3), bl.reshape(1)]).reshape(1, 4).astype(
        np.float32
    )

    xh = _round11(x)

    def to_T(a):
        a = a.reshape(NCORES, NBT, 128, T, KC, 128)
        return np.ascontiguousarray(a.transpose(0, 3, 1, 4, 5, 2))

    xhT = to_T(xh)
    maps = [
        dict(xhT=xhT[c], wh=wh, b_wc=b_wc, wlb=wlb) for c in range(NCORES)
    ]
    if scheme == "A":
        wl16 = np.ascontiguousarray(
            (WT - Wh).astype(ml_dtypes.bfloat16).reshape(KC, 128, 3, C)
        )
        xh16T = xhT.astype(ml_dtypes.bfloat16)
        for c in range(NCORES):
            maps[c]["xh16T"] = xh16T[c]
            maps[c]["wl16"] = wl16
    else:
        # fp8e5 correction: (e*SE)@(WT/SE) + (xh/SX)@(F*SX), contraction 1024
        e = (x - xh).astype(np.float32)
        F = (WT - Wh).astype(np.float32)
        lhs_cat = np.concatenate(
            [e * SE, xh * (1.0 / SX)], axis=-1
        ).astype(ml_dtypes.float8_e5m2)  # [B, T, 1024]
        rhs_cat = np.concatenate(
            [WT * (1.0 / SE), F * SX], axis=0
        ).astype(ml_dtypes.float8_e5m2)  # [1024, 3, C]
        # wcorr [KCP, 128, 2, 3, C]: row r = kp*256 + i*128 + p
        wcorr = np.ascontiguousarray(rhs_cat.reshape(KCP, 2, 128, 3, C).transpose(
            0, 2, 1, 3, 4
        ))
        # xcorr [cores, T, NBT, KCP, 128, 2, 128]: same row mapping
        a = lhs_cat.reshape(NCORES, NBT, 128, T, KCP, 2, 128)
        xcorr = np.ascontiguousarray(a.transpose(0, 3, 1, 4, 6, 5, 2))
        for c in range(NCORES):
            maps[c]["xcorr"] = xcorr[c]
            maps[c]["wcorr"] = wcorr
    return maps


def _postprocess_out(res_map):
    o = res_map["spk"]
    if o.dtype == np.uint8:
        return o.astype(np.float32)
    return o


SCHEME = "B"


def kernel(**inputs):
    nc = _get_nc2(SCHEME)
    in_maps = _prepare_in_maps2(inputs, scheme=SCHEME)
    res = bass_utils.run_bass_kernel_spmd(nc, in_maps, core_ids=list(range(NCORES)))
    return np.concatenate([_postprocess_out(r) for r in res.results], axis=0)


if __name__ == "__main__":
    rng = np.random.default_rng(0)
    s_in = 1.0 / np.sqrt(IN)
    s3 = 1.0 / np.sqrt(3.0)
    ins = dict(
        x=rng.standard_normal((B, T, IN)).astype(np.float32),
        W1=rng.uniform(-s_in, s_in, (C, IN)).astype(np.float32),
        b1=rng.uniform(-s_in, s_in, (C,)).astype(np.float32),
        W2=rng.uniform(-s_in, s_in, (C, IN)).astype(np.float32),
        b2=rng.uniform(-s_in, s_in, (C,)).astype(np.float32),
        W3=rng.uniform(-s_in, s_in, (C, IN)).astype(np.float32),
        b3=rng.uniform(-s_in, s_in, (C,)).astype(np.float32),
        Wl=rng.uniform(-s3, s3, (1, 3)).astype(np.float32),
        bl=rng.uniform(-s3, s3, (1,)).astype(np.float32),
        wins=T,
    )
    out = kernel(**ins)

    # numpy reference
    p = [
        (ins["x"].reshape(B * T, IN) @ ins[f"W{k+1}"].T + ins[f"b{k+1}"]).reshape(
            B, T, C
        )
        for k in range(3)
    ]
    mem = np.zeros((B, C, 4), np.float32)
    spk = np.zeros((B, C, 4), np.float32)
    exp = np.zeros((B, T, C, 4), np.float32)
    for t in range(T):
        inner = mem[..., :3] @ ins["Wl"][0] + ins["bl"][0]
        ia = np.stack([p[0][:, t], p[1][:, t], p[2][:, t], inner], axis=-1)
        mem = mem * np.float32(0.2) * (1.0 - spk) + ia
        spk = (mem > 0.8).astype(np.float32)
        exp[:, t] = spk
    exp = exp.reshape(B, T, C * 4)
    rel = np.linalg.norm(out - exp) / np.linalg.norm(exp)
    print("out", out.shape, out.dtype, "density", out.mean())
    print("rel err vs numpy fp32:", rel, "nflips", np.abs(out - exp).sum())

